# revision 24
# baseline (speedup 1.0000x reference)
"""Trainium2 Bass kernel for 3D volume attention (b=2, x=y=z=16, c=64,
heads=4, dim_head=32, qk-standardize over sequence, scale=16).

Sharding: batch*heads = 8 (b,h) pairs -> 8 NeuronCores, one pair per core.
Host pre-transposes x and pre-slices per-head weights; host sums the 4
head-partials per batch (pure unshard-reduce) and reshapes.

Per-core pipeline (s=4096, d=32):
  prologue: f32r projections -> raw q/k (4x replicated via replicated
            weights) + v^T; standardize (+x16 on q) with one-Newton rsqrt;
            build bf16 attention operands:
              qA/kA [128,s]  bf16 replicas (pass A)
              qP/kP [97,s]   bf16 hi/lo pairs (~f32-precision logits via
                             qhi*khi + qhi*klo + qlo*khi), row 96 = aug
              vaug  [128,33] per j-block: [v | 1]
  pass A  (S[i,j] layout): 4-way row-tile-packed bf16 matmuls -> PSUM,
          DVE reduce_max -> per-row max mhat -> PE transpose -> row 96 of qP
  pass B  (S^T[j,i] layout): K=97 bf16 matmul, psum = s - mhat
          -> ACT exp -> bf16 P^T [128,1024]
  AV:     P^T @ [v | 1] accumulation -> out^T[33,1024] and softmax denom l
  out:    1/l via ACT ln+exp(-x), broadcast by rank-1 matmul, normalize,
          project with [w_out_h ; b_out/4], DMA out^T.
"""
import os
import sys
from contextlib import ExitStack

import numpy as np

_PROBLEM_DIR = os.path.dirname(os.path.abspath(__file__))
if _PROBLEM_DIR not in sys.path:
    sys.path.insert(0, _PROBLEM_DIR)

import concourse.bass as bass
import concourse.tile as tile
from concourse import bacc, mybir
from concourse.bass_utils import run_bass_kernel_spmd

F32 = mybir.dt.float32
F32R = mybir.dt.float32r
BF16 = mybir.dt.bfloat16
AF = mybir.ActivationFunctionType
ALU = mybir.AluOpType

HEADS = 4
DH = 32          # dim head
CIN = 64         # input channels
S = 4096         # sequence (16^3)
SCALE = 16.0
EPS = 1e-5
NB = S // 128    # 32 j/i blocks
NCH = 4          # i chunks
CHUNK = 1024
KP = 97          # 3*32 pair rows + 1 aug row

_compiled = None
STAGE = int(os.environ.get("STAGE", "4"))


def _build():
    nc = bacc.Bacc("TRN2", target_bir_lowering=False, debug=False, num_devices=8)
    xT_d = nc.dram_tensor("xT", [CIN, S], F32, kind="ExternalInput").ap()
    wq_d = nc.dram_tensor("wq", [CIN, 128], F32, kind="ExternalInput").ap()
    wk_d = nc.dram_tensor("wk", [CIN, 128], F32, kind="ExternalInput").ap()
    wv_d = nc.dram_tensor("wv", [CIN, DH], F32, kind="ExternalInput").ap()
    wo_d = nc.dram_tensor("wo", [DH + 1, CIN], F32, kind="ExternalInput").ap()
    out_d = nc.dram_tensor("out", [CIN, S], F32, kind="ExternalOutput").ap()

    with tile.TileContext(nc) as tc, ExitStack() as ctx:
        per = ctx.enter_context(tc.tile_pool(name="per", bufs=1))

        # ---- persistent SBUF ----
        wo_r = per.tile([DH + 1, CIN], F32R)
        qA = per.tile([128, S], BF16)      # 4 replicated bands of qhat*16
        kA = per.tile([128, S], BF16)      # 4 replicated bands of khat
        qP = per.tile([KP, S], BF16)       # [qhi; qhi; qlo; mhat-row]
        kP = per.tile([KP, S], BF16)       # [khi; klo; khi; -1]
        vaug = per.tile([128, 33 * NB], BF16)
        yT = per.tile([CIN, S], F32)
        ident = per.tile([128, 128], F32)
        ones33_f = per.tile([1, 33], F32)
        ones33 = per.tile([1, 33], F32R)

        with tc.tile_pool(name="prow", bufs=1) as prow, \
             tc.tile_pool(name="props", bufs=2, space="PSUM") as props:
            xT = prow.tile([CIN, S], F32)
            nc.sync.dma_start(xT[:], xT_d[:])
            wq = prow.tile([CIN, 128], F32)
            wk = prow.tile([CIN, 128], F32)
            wv = prow.tile([CIN, DH], F32)
            nc.sync.dma_start(wq[:], wq_d[:])
            nc.sync.dma_start(wk[:], wk_d[:])
            nc.sync.dma_start(wv[:], wv_d[:])
            wo = prow.tile([DH + 1, CIN], F32)
            nc.sync.dma_start(wo[:], wo_d[:])
            nc.vector.tensor_copy(wo_r[:], wo[:])

            xTr = prow.tile([CIN, S], F32R)
            nc.vector.tensor_copy(xTr[:], xT[:])
            wq_r = prow.tile([CIN, 128], F32R)
            wk_r = prow.tile([CIN, 128], F32R)
            wv_r = prow.tile([CIN, DH], F32R)
            nc.vector.tensor_copy(wq_r[:], wq[:])
            nc.vector.tensor_copy(wk_r[:], wk[:])
            nc.vector.tensor_copy(wv_r[:], wv[:])

            # ---- projections: q/k raw (4x replicated via replicated weights) ----
            qraw = prow.tile([128, S], F32)
            kraw = prow.tile([128, S], F32)
            sx_q = prow.tile([128, 2], F32)
            sx_k = prow.tile([128, 2], F32)
            for half in range(2):
                for dst_raw, w_r, sx in ((qraw, wq_r, sx_q), (kraw, wk_r, sx_k)):
                    pp = props.tile([128, 2048], F32, name=f"pp{half}", tag="pp")
                    for n in range(4):
                        sl = bass.ds(2048 * half + 512 * n, 512)
                        nc.tensor.matmul(pp[:, bass.ts(n, 512)], w_r[:], xTr[:, sl],
                                         start=True, stop=True)
                    nc.scalar.activation(dst_raw[:, bass.ts(half, 2048)], pp[:],
                                         AF.Copy, accum_out=sx[:, half:half + 1])

            # ---- v projection: vT = wv.T @ x, then bank-aligned transposes ----
            nc.vector.memset(vaug[:], 1.0)
            vT = prow.tile([DH, S], F32)
            for half in range(2):
                pv = props.tile([128, 2048], F32, name=f"pv{half}", tag="pp")
                for n in range(4):
                    nc.tensor.matmul(pv[0:DH, bass.ts(n, 512)], wv_r[:],
                                     xTr[:, bass.ds(2048 * half + 512 * n, 512)],
                                     start=True, stop=True)
                nc.scalar.copy(vT[:, bass.ts(half, 2048)], pv[0:DH, :])
            from concourse.masks import make_identity
            make_identity(nc, ident[:])
            for g in range(8):
                pv = props.tile([128, 2048], F32, name=f"pvt{g}", tag="pp")
                for t in range(4):
                    jb = 4 * g + t
                    nc.tensor.transpose(pv[:, bass.ds(512 * t, DH)],
                                        vT[:, bass.ts(jb, 128)], ident[0:DH, 0:DH])
                for t in range(4):
                    jb = 4 * g + t
                    nc.scalar.copy(vaug[:, bass.ds(33 * jb, DH)],
                                   pv[:, bass.ds(512 * t, DH)])

            # ---- stats ----
            sq_q = prow.tile([128, 2], F32)
            sq_k = prow.tile([128, 2], F32)
            junk = prow.tile([128, 2048], BF16)
            for half in range(2):
                nc.scalar.activation(junk[:], qraw[:, bass.ts(half, 2048)], AF.Square,
                                     accum_out=sq_q[:, half:half + 1])
                nc.scalar.activation(junk[:], kraw[:, bass.ts(half, 2048)], AF.Square,
                                     accum_out=sq_k[:, half:half + 1])

            def finish_stats(sx, sq, fold):
                mu = prow.tile([128, 1], F32, name=f"mu{fold}")
                nc.vector.tensor_tensor(out=mu[:], in0=sx[:, 0:1], in1=sx[:, 1:2],
                                        op=ALU.add)
                nc.vector.tensor_scalar_mul(mu[:], mu[:], 1.0 / S)
                ex2 = prow.tile([128, 1], F32, name=f"ex2{fold}")
                nc.vector.tensor_tensor(out=ex2[:], in0=sq[:, 0:1], in1=sq[:, 1:2],
                                        op=ALU.add)
                nc.vector.tensor_scalar_mul(ex2[:], ex2[:], 1.0 / S)
                musq = prow.tile([128, 1], F32, name=f"musq{fold}")
                nc.vector.tensor_tensor(out=musq[:], in0=mu[:], in1=mu[:], op=ALU.mult)
                vareps = prow.tile([128, 1], F32, name=f"vareps{fold}")
                nc.vector.tensor_tensor(out=vareps[:], in0=ex2[:], in1=musq[:],
                                        op=ALU.subtract)
                nc.vector.tensor_scalar_add(vareps[:], vareps[:], EPS)
                sq_t = prow.tile([128, 1], F32, name=f"sqt{fold}")
                nc.scalar.activation(sq_t[:], vareps[:], AF.Sqrt)
                r0 = prow.tile([128, 1], F32, name=f"r0{fold}")
                nc.vector.reciprocal(r0[:], sq_t[:])
                r0sq = prow.tile([128, 1], F32, name=f"r0sq{fold}")
                nc.vector.tensor_tensor(out=r0sq[:], in0=r0[:], in1=r0[:], op=ALU.mult)
                h = prow.tile([128, 1], F32, name=f"h{fold}")
                nc.vector.tensor_tensor(out=h[:], in0=r0sq[:], in1=vareps[:],
                                        op=ALU.mult)
                w = prow.tile([128, 1], F32, name=f"w{fold}")
                nc.vector.tensor_scalar(out=w[:], in0=h[:], scalar1=-0.5, scalar2=1.5,
                                        op0=ALU.mult, op1=ALU.add)
                rstd = prow.tile([128, 1], F32, name=f"rstd{fold}")
                nc.vector.tensor_tensor(out=rstd[:], in0=r0[:], in1=w[:], op=ALU.mult)
                if fold != 1.0:
                    nc.vector.tensor_scalar_mul(rstd[:], rstd[:], fold)
                return mu, rstd

            mu_q, rstd_q = finish_stats(sx_q, sq_q, SCALE)
            mu_k, rstd_k = finish_stats(sx_k, sq_k, 1.0)

            # ---- bf16 replicas for pass A ----
            nc.vector.tensor_scalar(out=qA[:], in0=qraw[:], scalar1=mu_q[:],
                                    scalar2=rstd_q[:], op0=ALU.subtract, op1=ALU.mult)
            nc.vector.tensor_scalar(out=kA[:], in0=kraw[:], scalar1=mu_k[:],
                                    scalar2=rstd_k[:], op0=ALU.subtract, op1=ALU.mult)

            # ---- hi/lo pair tiles for pass B ----
            qf = prow.tile([DH, S], F32)
            kf = prow.tile([DH, S], F32)
            nc.vector.tensor_scalar(out=qf[:], in0=qraw[0:DH, :], scalar1=mu_q[0:DH, :],
                                    scalar2=rstd_q[0:DH, :], op0=ALU.subtract,
                                    op1=ALU.mult)
            nc.vector.tensor_scalar(out=kf[:], in0=kraw[0:DH, :], scalar1=mu_k[0:DH, :],
                                    scalar2=rstd_k[0:DH, :], op0=ALU.subtract,
                                    op1=ALU.mult)
            # hi parts = bf16 rounds (same values as qA/kA rows 0:32)
            nc.sync.dma_start(qP[0:DH, :], qA[0:DH, :])
            nc.sync.dma_start(qP[DH:2 * DH, :], qA[0:DH, :])
            nc.sync.dma_start(kP[0:DH, :], kA[0:DH, :])
            nc.sync.dma_start(kP[2 * DH:3 * DH, :], kA[0:DH, :])
            # lo parts = round(f32 - hi)  (gpsimd keeps DVE free; gpsimd lanes
            # are partition-locked so compute at partitions 0:32, DMA-shift)
            qlo_t = prow.tile([DH, S], BF16)
            klo_t = prow.tile([DH, S], BF16)
            nc.gpsimd.tensor_tensor(out=qlo_t[:], in0=qf[:], in1=qA[0:DH, :],
                                    op=ALU.subtract)
            nc.gpsimd.tensor_tensor(out=klo_t[:], in0=kf[:], in1=kA[0:DH, :],
                                    op=ALU.subtract)
            nc.sync.dma_start(qP[2 * DH:3 * DH, :], qlo_t[:])
            nc.sync.dma_start(kP[DH:2 * DH, :], klo_t[:])
            # kP row 96 = -1 (junk is bf16; -1 exact)
            nc.vector.memset(junk[:], -1.0)
            for half in range(2):
                nc.vector.tensor_copy(kP[96:97, bass.ts(half, 2048)],
                                      junk[96:97, :])
            nc.vector.memset(ones33_f[:], 1.0)
            nc.vector.tensor_copy(ones33[:], ones33_f[:])

        # ================= main loop =================
        with tc.tile_pool(name="uni", bufs=3, space="PSUM") as uni_pool, \
             tc.tile_pool(name="psAV", bufs=1, space="PSUM") as psAV_pool, \
             tc.tile_pool(name="mcolp", bufs=2) as mcol_pool, \
             tc.tile_pool(name="ptp", bufs=3) as pt_pool, \
             tc.tile_pool(name="ppn", bufs=2) as pn_pool:

            def emit_passA_block(ib):
                mparts = mcol_pool.tile([128, 4], F32, name=f"mp{ib}", tag="mparts")
                for quarter in range(4):
                    psA = uni_pool.tile([128, 1024], F32, name=f"psA{ib}_{quarter}",
                                        tag="uni")
                    for r in range(2):
                        nc.tensor.matmul(
                            psA[:, bass.ts(r, 512)],
                            qA[bass.ts(r, 32), bass.ts(ib, 128)],
                            kA[bass.ts(r, 32), bass.ds(1024 * quarter + 512 * r, 512)],
                            start=True, stop=True,
                            tile_position=(32 * r, 0),
                        )
                    nc.vector.reduce_max(mparts[:, quarter:quarter + 1], psA[:],
                                         axis=mybir.AxisListType.X)
                m01 = mcol_pool.tile([128, 1], F32, name=f"m01_{ib}", tag="m01")
                nc.vector.tensor_tensor(out=m01[:], in0=mparts[:, 0:1],
                                        in1=mparts[:, 1:2], op=ALU.max)
                m23 = mcol_pool.tile([128, 1], F32, name=f"m23_{ib}", tag="m23")
                nc.vector.tensor_tensor(out=m23[:], in0=mparts[:, 2:3],
                                        in1=mparts[:, 3:4], op=ALU.max)
                mcol = mcol_pool.tile([128, 1], F32, name=f"mcol{ib}", tag="mcol")
                nc.vector.tensor_tensor(out=mcol[:], in0=m01[:], in1=m23[:],
                                        op=ALU.max)
                # transpose mhat column -> bf16 row at partition 96 of qP
                psmT = uni_pool.tile([128, 128], F32, name=f"psmT{ib}", tag="uni")
                nc.tensor.transpose(psmT[0:1, :], mcol[:], ident[:])
                mrow0 = mcol_pool.tile([1, 128], BF16, name=f"mrow0_{ib}",
                                       tag="mrow0")
                nc.scalar.copy(mrow0[:], psmT[0:1, :])
                stag = mcol_pool.tile([KP, 128], BF16, name=f"stag{ib}", tag="stag")
                nc.sync.dma_start(stag[96:97, :], mrow0[:])
                nc.scalar.copy(qP[96:97, bass.ts(ib, 128)], stag[96:97, :])

            # prologue-peel: pass A for chunk 0
            for r_ib in range(8):
                emit_passA_block(r_ib)

            for ch in range(NCH):
                if STAGE <= 2:
                    if ch == 0:
                        nc.vector.memset(yT[:], 0.0)
                    for r_ib in range(8 * ch, 8 * ch + 8):
                        if r_ib >= 8:
                            emit_passA_block(r_ib)
                    continue
                # ---------- pass B + AV ----------
                avh = [psAV_pool.tile([33, 512], F32, name=f"av{ch}_{h}", tag=f"av{h}")
                       for h in range(2)]
                for jb in range(NB):
                    # software-pipeline: emit next chunk's pass-A blocks
                    if jb % 4 == 0 and ch + 1 < NCH:
                        emit_passA_block(8 * (ch + 1) + jb // 4)
                    psB = uni_pool.tile([128, CHUNK], F32, name=f"psB{ch}_{jb}",
                                        tag="uni")
                    for hf in range(2):
                        nc.tensor.matmul(psB[:, bass.ts(hf, 512)],
                                         kP[:, bass.ts(jb, 128)],
                                         qP[:, bass.ds(CHUNK * ch + 512 * hf, 512)],
                                         start=True, stop=True)
                    pt = pt_pool.tile([128, CHUNK], BF16, name=f"pt{ch}_{jb}",
                                      tag="pt")
                    nc.scalar.activation(pt[:], psB[:], AF.Exp)
                    if STAGE == 3:
                        nc.vector.reduce_max(yT[0:128, ch:ch + 1] if False else mparts[:, 0:1], pt[:], axis=mybir.AxisListType.X)
                        continue
                    for hf in range(2):
                        nc.tensor.matmul(avh[hf][:],
                                         vaug[:, bass.ds(33 * jb, 33)],
                                         pt[:, bass.ts(hf, 512)],
                                         start=(jb == 0), stop=(jb == NB - 1))

                if STAGE == 3:
                    if ch == 0:
                        nc.vector.memset(yT[:], 0.0)
                    continue
                # ---------- normalize + output projection ----------
                lnl = pn_pool.tile([1, CHUNK], F32, name=f"lnl{ch}", tag="lnl")
                linv = pn_pool.tile([1, CHUNK], F32R, name=f"linv{ch}", tag="linv")
                pl = uni_pool.tile([33, CHUNK], F32, name=f"pl{ch}", tag="uni")
                linv_rep = pn_pool.tile([33, CHUNK], F32, name=f"linvrep{ch}",
                                        tag="linvrep")
                pn = pn_pool.tile([33, CHUNK], F32R, name=f"pn{ch}", tag="pn")
                psY = uni_pool.tile([64, CHUNK], F32, name=f"psY{ch}", tag="uni")
                for half in range(2):
                    hs = bass.ts(half, 512)
                    nc.scalar.activation(lnl[:, hs], avh[half][32:33, :], AF.Ln)
                    nc.scalar.activation(linv[:, hs], lnl[:, hs], AF.Exp, scale=-1.0)
                    nc.tensor.matmul(pl[:, hs], ones33[:], linv[:, hs],
                                     start=True, stop=True)
                    nc.scalar.copy(linv_rep[:, hs], pl[:, hs])
                    nc.vector.tensor_tensor(out=pn[:, hs], in0=avh[half][:],
                                            in1=linv_rep[:, hs], op=ALU.mult)
                    nc.tensor.matmul(psY[:, hs], wo_r[:], pn[:, hs],
                                     start=True, stop=True)
                nc.scalar.copy(yT[:, bass.ts(ch, CHUNK)], psY[:])

            nc.sync.dma_start(out_d[:], yT[:])

    nc.compile()
    return nc


def _get_compiled():
    global _compiled
    if _compiled is None:
        _compiled = _build()
    return _compiled


def kernel(input, w_qkv, w_out, b_out):
    input = np.asarray(input, dtype=np.float32)
    w_qkv = np.asarray(w_qkv, dtype=np.float32)
    w_out = np.asarray(w_out, dtype=np.float32)
    b_out = np.asarray(b_out, dtype=np.float32)
    b, x, y, z, c = input.shape
    assert (b, x, y, z, c) == (2, 16, 16, 16, 64)
    hid = HEADS * DH

    in_maps = []
    for core in range(8):
        bb, h = divmod(core, HEADS)
        xT = np.ascontiguousarray(input[bb].reshape(S, CIN).T)
        wq = np.tile(w_qkv[:, h * DH:(h + 1) * DH], (1, 4))
        wk = np.tile(w_qkv[:, hid + h * DH: hid + (h + 1) * DH], (1, 4))
        wv = np.ascontiguousarray(w_qkv[:, 2 * hid + h * DH: 2 * hid + (h + 1) * DH])
        wo = np.vstack([w_out[h * DH:(h + 1) * DH, :], b_out[None, :] / HEADS])
        in_maps.append({
            "xT": xT,
            "wq": np.ascontiguousarray(wq),
            "wk": np.ascontiguousarray(wk),
            "wv": wv,
            "wo": np.ascontiguousarray(wo),
        })

    global _last_in_maps
    _last_in_maps = in_maps
    nc = _get_compiled()
    res = run_bass_kernel_spmd(nc, in_maps, core_ids=list(range(8)))
    out = np.zeros((b, S, CIN), dtype=np.float32)
    for core in range(8):
        bb = core // HEADS
        out[bb] += res.results[core]["out"].T
    return out.reshape(b, x, y, z, CIN)


if __name__ == "__main__":
    rng = np.random.default_rng(0)
    inp = rng.standard_normal((2, 16, 16, 16, 64), dtype=np.float32)
    wqkv = rng.standard_normal((64, 384), dtype=np.float32) / 8.0
    wout = rng.standard_normal((128, 64), dtype=np.float32) / np.sqrt(128)
    bout = np.zeros(64, dtype=np.float32)
    o = kernel(inp, wqkv, wout, bout)
    print("kernel output shape:", o.shape)
